# revision 59
# baseline (speedup 1.0000x reference)
"""Trainium2 Bass kernel for nn_MeanAddCelltype (GNN mean-aggregate + residual + MLP).

Reference semantics (N=8192 nodes, K=16 neighbors, D=512):
    idx  = top_k(fake_edge_mask, 16).indices          # per-row indices of the 16 ones
    res  = mean(x[idx], axis=1)                       # neighbor mean
    out  = relu((x + res) @ W1 + b1) @ W2 + b2

Because fake_edge_mask has exactly 16 ones per row and the neighbor sum is
permutation-invariant, res == (fake_edge_mask @ x) / 16 exactly. We compute
the aggregation as a block-sparse mask matmul on the tensor engine instead of
a top_k + gather.

Sharding: rows (nodes) are split across 8 cores, 1024 rows each; the MLP
weights are replicated. No collectives.

Block sparsity: the contraction over source nodes j (64 chunks of 128) only
matters for chunks where this core's mask slice has any nonzero. The host
scans block occupancy (CSR-style metadata, indices only).

Row rotation: each core relabels source nodes j' = (j + c*1024) mod N and
applies the same permutation to the mask rows and the x rows it contracts
against — a content-preserving relayout that leaves the output unchanged.
This puts every core's own-diagonal blocks (and, for neighborhood-local
graphs, all its occupied blocks) at block indices 0..W-1, so phase 1 reads a
statically-addressed packed window with a few large direct DMAs. Occupied
blocks beyond the window (arbitrary masks) are fetched by indirect row
gathers driven by a host-provided offset table; cores with fewer extra
blocks point the pad entries at an appended all-zero block.

Residual folding: the host adds 16*I (exact in fp16) on the core's own rows'
diagonal, which after rotation lies in window blocks 0..7. With x pre-scaled
by 1/16, the block matmul then accumulates res + x = hidden directly in
PSUM, so no separate residual add is needed.

Layout trick: all activations are kept feature-major ("transposed", [D, rows])
so every matmul consumes natural-layout operands:
    hiddenT [512,1024] = sum_{k in blocks} (x/16)[k].T-part @ (maskT+16I)[k]
    h1T  [1024,1024]   = relu(W1.T-part @ hiddenT + b1)
    outT [512,1024]    = W2.T-part @ h1T + b2
All matmul operands are fp16 (mask 0/1 and 16 are exact in fp16; x/W rounding
gives ~5e-4 rel err); accumulation is fp32 in PSUM. The host transposes
per-core mask/x slices and transposes the per-core outputs back.
"""

import os
import numpy as np

import concourse.bass as bass
import concourse.bacc as bacc
import concourse.mybir as mybir
import concourse.tile as tile
from concourse.bass_utils import run_bass_kernel_spmd

N = 8192
D_IN = 512
D_HID = 1024
D_OUT = 512
N_NEIGHS = 16
N_CORES = 8
ROWS = N // N_CORES          # 1024 rows per core
KCH = N // 128               # 64 possible contraction chunks over source nodes
OWN = ROWS // 128            # 8 diagonal blocks per core
WMAX = 16                    # max static-window size (blocks)
F16 = mybir.dt.float16
F32 = mybir.dt.float32
I32 = mybir.dt.int32
AF = mybir.ActivationFunctionType

# Results of the last hardware run (for test harness introspection).
LAST = {}

_PROGRAMS = {}


def _install_ntff_hook():
    """Best-effort shim for NTFF profiling under axon.

    This image's ``antenv`` package lacks the ``axon_hooks`` module that
    ``run_bass_kernel_spmd(trace=True)`` consults, but the actual ctypes
    profiling driver exists in ``trn_agent_boot.trn_boot``. Register it
    ourselves, and keep profile artifacts local (no remote upload).
    Failures here only disable tracing, never the run.
    """
    import sys
    import types
    try:
        try:
            from antenv import axon_hooks  # noqa: F401
            return
        except ImportError:
            pass
        import antenv
        from trn_agent_boot.trn_boot import _ntff_profile_via_ctypes
        hook = _ntff_profile_via_ctypes("/opt/axon/libaxon_pjrt.so")
        mod = types.ModuleType("antenv.axon_hooks")
        mod._hook = hook
        mod.set_axon_ntff_profile_hook = lambda h: setattr(mod, "_hook", h)
        mod.get_axon_ntff_profile_hook = lambda: mod._hook
        sys.modules["antenv.axon_hooks"] = mod
        antenv.axon_hooks = mod
        import concourse.bass_utils as bu
        bu.upload_artifacts = lambda tmpdir: "local://" + str(tmpdir)
    except Exception as e:  # pragma: no cover
        print(f"ntff hook install failed ({e!r}); tracing disabled", file=sys.stderr)


def _window_splits(w):
    """Split window blocks into DMA chunks, small first so PE starts early."""
    splits, lo, size = [], 0, 1
    while lo < w:
        hi = min(w, lo + size)
        splits.append((lo, hi))
        lo = hi
        size = 4
    return splits


def _build_program(w, n_extra, ranges):
    """Per-core Bass/Tile program (same BIR on all 8 cores): ``w`` static
    window blocks + ``n_extra`` gathered blocks in the phase-1 contraction.

    ``ranges[b]`` (window blocks only) is the (lo, hi) column range — the
    union over cores of this block's nonzero mask columns. Extra blocks
    always run full width."""
    nc = bacc.Bacc("TRN2", target_bir_lowering=False, debug=False,
                   num_devices=N_CORES)

    # Packed static window, partition-major. The mask window is packed by
    # each block's nonzero column range (``ranges[b] = (lo, hi)``); the x
    # window is dense: [p, b*D_IN + j].
    wid = [hi - lo for lo, hi in ranges]
    poff = np.concatenate([[0], np.cumsum(wid)]).tolist()        # pack offsets
    mw = nc.dram_tensor("mw", [128, max(poff[-1], 1)], F16, kind="ExternalInput")
    xw = nc.dram_tensor("xw", [128, w * D_IN], F16, kind="ExternalInput")
    if n_extra:
        # Full rotated tensors (+ one all-zero pad block) for row gathers.
        mt = nc.dram_tensor("mt", [N + 128, ROWS], F16, kind="ExternalInput")
        xs = nc.dram_tensor("xs", [N + 128, D_IN], F16, kind="ExternalInput")
        of = nc.dram_tensor("of", [128, n_extra], I32, kind="ExternalInput")
    w1 = nc.dram_tensor("w1", [D_IN, D_HID], F16, kind="ExternalInput")
    w2 = nc.dram_tensor("w2", [D_HID, D_OUT], F16, kind="ExternalInput")
    b1 = nc.dram_tensor("b1", [D_HID // 128, 128, 1], F32, kind="ExternalInput")
    b2 = nc.dram_tensor("b2", [D_OUT // 128, 128, 1], F32, kind="ExternalInput")
    ot = nc.dram_tensor("ot", [D_OUT, ROWS], F32, kind="ExternalOutput")    # outT

    w1_v = w1.ap().rearrange("(n p) m -> n p m", p=128)   # [4, 128, 1024]
    w2_v = w2.ap().rearrange("(n p) m -> n p m", p=128)   # [8, 128, 512]
    ot_v = ot.ap().rearrange("(n p) m -> n p m", p=128)   # [4, 128, 1024]

    # Per-bank-half accumulation-group stop bookkeeping: the last block
    # contributing to half h carries stop for all 4 d-accumulators' h-bank.
    blocks_h = {h: [b for b in range(w)
                    if ranges[b][0] < (h + 1) * 512 and ranges[b][1] > h * 512]
                + list(range(w, w + n_extra)) for h in (0, 1)}

    with tile.TileContext(nc) as tc:
        with (
            tc.tile_pool(name="const", bufs=1) as const,
            tc.tile_pool(name="io", bufs=3) as io,
            tc.tile_pool(name="acts", bufs=1) as acts,
            tc.tile_pool(name="acc", bufs=8, space=bass.MemorySpace.PSUM) as acc,
        ):
            # --- phase 1: hiddenT = sum_k (x/16)[k].T @ (maskT+16I)[k] ---
            # 4 PSUM accumulators, one per d-chunk, each [128, 1024] fp32 =
            # two PSUM banks. Block matmuls write partial, possibly
            # overlapping column ranges, and a matmul's PSUM range must be
            # all-pending or all-initialized — so each accumulator is armed
            # first by a full-width matmul against a zeroed moving operand
            # (start=True). These arming matmuls have no DMA dependency and
            # run during the startup window (also warming the PE clock).
            ps = [acc.tile([128, 512], F32, tag="ps", name=f"ps{g}")
                  for g in range(8)]
            zt = acts.tile([128, 512], F16, name="zt")
            nc.gpsimd.memset(zt[:], 0.0)
            armed = [False, False]

            def ensure_armed(h):
                # Arm the 4 d-banks of half h right before the first block
                # that touches it, so the in-order PE FIFO doesn't hold the
                # first data matmuls behind arming work for the other half.
                if not armed[h]:
                    armed[h] = True
                    for d in range(4):
                        nc.tensor.matmul(ps[d * 2 + h][:], zt[:, :128], zt[:],
                                         start=True, stop=False,
                                         skip_group_check=True)

            ensure_armed(0)

            def block_matmuls(b, mk, xk, mo, xo):
                # Window block: mk holds packed columns [lo, hi) at offset
                # mo. Extra block: mk holds all 1024 columns. Matmuls may
                # not cross the PSUM bank boundary, so clip at column 512.
                (blo, bhi) = ranges[b] if b < w else (0, ROWS)
                segs = [(h, max(blo, h * 512), min(bhi, (h + 1) * 512))
                        for h in range(2)]
                segs = [(h, lo, hi) for (h, lo, hi) in segs if lo < hi]
                for (h, _, _) in segs:
                    ensure_armed(h)
                for d in range(4):
                    lhsT = xk[:, xo + d * 128:xo + (d + 1) * 128]
                    for (h, lo, hi) in segs:
                        nc.tensor.matmul(
                            ps[d * 2 + h][:, lo - h * 512:hi - h * 512],
                            lhsT,
                            mk[:, mo + (lo - blo):mo + (hi - blo)],
                            start=False,
                            stop=(b == blocks_h[h][-1]),
                            skip_group_check=True,
                        )

            if n_extra:
                of_sb = const.tile([128, n_extra], I32, name="of_sb")
                nc.sync.dma_start(of_sb[:], of.ap()[:])

            for s, (lo, hi) in enumerate(_window_splits(w)):
                nb = hi - lo
                mwid = poff[hi] - poff[lo]
                xk = io.tile([128, nb * D_IN], F16, tag="xk", name=f"xk{s}")
                mk = io.tile([128, max(mwid, 1)], F16, tag="mk", name=f"mk{s}")
                # Launch x loads from the sync queue and mask loads from the
                # gpsimd queue so the per-sequencer launch overheads overlap.
                nc.sync.dma_start(xk[:], xw.ap()[:, lo * D_IN:hi * D_IN])
                if mwid:
                    nc.gpsimd.dma_start(mk[:], mw.ap()[:, poff[lo]:poff[hi]])
                for b in range(lo, hi):
                    block_matmuls(b, mk, xk, poff[b] - poff[lo], (b - lo) * D_IN)

            for e in range(n_extra):
                mk = io.tile([128, ROWS], F16, tag="mke", name=f"mke{e}")
                xk = io.tile([128, D_IN], F16, tag="xke", name=f"xke{e}")
                nc.gpsimd.indirect_dma_start(
                    out=mk[:], out_offset=None, in_=mt.ap(),
                    in_offset=bass.IndirectOffsetOnAxis(ap=of_sb[:, e:e + 1], axis=0),
                )
                nc.gpsimd.indirect_dma_start(
                    out=xk[:], out_offset=None, in_=xs.ap(),
                    in_offset=bass.IndirectOffsetOnAxis(ap=of_sb[:, e:e + 1], axis=0),
                )
                block_matmuls(w + e, mk, xk, 0, 0)

            # --- resident constants (needed from phase 2 on) -------------
            w1_sb = []
            for i in range(4):
                t = const.tile([128, D_HID], F16, name=f"w1_{i}")
                nc.sync.dma_start(t[:], w1_v[i])
                w1_sb.append(t)
            w2_sb = []
            for i in range(8):
                t = const.tile([128, D_OUT], F16, name=f"w2_{i}")
                nc.sync.dma_start(t[:], w2_v[i])
                w2_sb.append(t)
            b1_sb = const.tile([128, 8], F32, name="b1_sb")
            for m in range(8):
                nc.sync.dma_start(b1_sb[:, m:m + 1], b1.ap()[m])
            b2_sb = const.tile([128, 4], F32, name="b2_sb")
            for m in range(4):
                nc.sync.dma_start(b2_sb[:, m:m + 1], b2.ap()[m])

            ensure_armed(1)     # degenerate masks may never touch half 1

            # --- phase 2: hT = fp16(psum) -------------------------------
            # (residual already folded in via the +16I diagonal). Split the
            # PSUM->SBUF casts across DVE and ACT.
            hT = [acts.tile([128, ROWS], F16, name=f"hT{d}") for d in range(4)]
            for h in range(2):
                for d in range(4):
                    dst = hT[d][:, h * 512:(h + 1) * 512]
                    if d % 2 == 0:
                        nc.vector.tensor_copy(dst, ps[d * 2 + h][:])
                    else:
                        nc.scalar.copy(dst, ps[d * 2 + h][:])

            # --- phase 3: h1T = relu(W1_part.T @ hT + b1) ----------------
            # 1024-wide moving operands (2 PSUM banks per matmul) halve the
            # LDWEIGHTS count; evacuation still runs per 512-col half.
            h1 = [acts.tile([128, ROWS], F16, name=f"h1_{m}") for m in range(8)]
            for m in range(8):
                for h in range(2):
                    pg = acc.tile([128, 512], F32, tag="ps", name=f"pg1_{m}_{h}")
                    for kd in range(4):
                        nc.tensor.matmul(
                            pg[:],
                            w1_sb[kd][:, m * 128:(m + 1) * 128],
                            hT[kd][:, h * 512:(h + 1) * 512],
                            start=(kd == 0),
                            stop=(kd == 3),
                        )
                    nc.scalar.activation(
                        h1[m][:, h * 512:(h + 1) * 512], pg[:],
                        AF.Relu, bias=b1_sb[:, m:m + 1],
                    )

            # --- phase 4: outT = W2_part.T @ h1T + b2 --------------------
            for o in range(4):
                ob = acts.tile([128, ROWS], F32, name=f"ob{o}")
                for h in range(2):
                    pg = acc.tile([128, 512], F32, tag="ps", name=f"pg2_{o}_{h}")
                    for kh in range(8):
                        nc.tensor.matmul(
                            pg[:],
                            w2_sb[kh][:, o * 128:(o + 1) * 128],
                            h1[kh][:, h * 512:(h + 1) * 512],
                            start=(kh == 0),
                            stop=(kh == 7),
                        )
                    nc.scalar.activation(
                        ob[:, h * 512:(h + 1) * 512], pg[:],
                        AF.Identity, bias=b2_sb[:, o:o + 1],
                    )
                    nc.sync.dma_start(ot_v[o][:, h * 512:(h + 1) * 512],
                                      ob[:, h * 512:(h + 1) * 512])

    nc.compile()
    return nc


def _get_program(key):
    if key not in _PROGRAMS:
        _PROGRAMS[key] = _build_program(*key)
    return _PROGRAMS[key]


def _pack(v):
    """[nb*128, fd] chunk-major -> [128, nb*fd] partition-major packing."""
    nb = v.shape[0] // 128
    return np.ascontiguousarray(
        v.reshape(nb, 128, v.shape[1]).transpose(1, 0, 2)).reshape(128, -1)


def _effective_mask(mask):
    """Reproduce top_k(mask, 16) selection semantics exactly: the reference
    gathers the 16 highest-valued columns per row with ties broken by
    ascending index. For rows with exactly 16 ones (the documented
    invariant) that is just the ones; rows that deviate select the
    lowest-index ones first, then the lowest-index zeros. No-op cost when
    every row has exactly 16 ones."""
    cnt = mask.sum(axis=1)
    bad = np.flatnonzero(cnt != N_NEIGHS)
    if not bad.size:
        return mask
    mask = mask.copy()
    for r in bad:
        ones = np.flatnonzero(mask[r])
        sel = ones[:N_NEIGHS]
        if sel.size < N_NEIGHS:
            zeros = np.flatnonzero(~mask[r])
            sel = np.concatenate([sel, zeros[:N_NEIGHS - sel.size]])
        row = np.zeros(mask.shape[1], dtype=bool)
        row[sel] = True
        mask[r] = row
    return mask


def _prepare_in_maps(x, fake_edge_mask, W1, b1, W2, b2):
    x = np.asarray(x, dtype=np.float32)
    mask = _effective_mask(np.asarray(fake_edge_mask).astype(bool))
    xs16 = (x * (1.0 / N_NEIGHS)).astype(np.float16)       # exact pow2 scale
    w1h = np.asarray(W1, dtype=np.float32).astype(np.float16)
    w2h = np.asarray(W2, dtype=np.float32).astype(np.float16)
    b1r = np.ascontiguousarray(
        np.asarray(b1, dtype=np.float32).reshape(D_HID // 128, 128, 1))
    b2r = np.ascontiguousarray(
        np.asarray(b2, dtype=np.float32).reshape(D_OUT // 128, 128, 1))

    # Occupied 128-row source blocks per core in ROTATED order (indices-only
    # metadata). Rotation: core c relabels source j -> (j - c*ROWS) mod N,
    # which is a left-rotation of blocks by c*OWN. The +16I diagonal then
    # occupies blocks 0..OWN-1 (always in-window).
    occ = mask.reshape(N_CORES, ROWS, KCH, 128).any(axis=(1, 3))
    win_c, extra_c = [], []
    for c in range(N_CORES):
        occ_rot = np.roll(occ[c], -c * OWN)
        idx = np.flatnonzero(occ_rot)
        in_win = idx[idx < WMAX]
        win_c.append(max(int(in_win.max()) + 1 if in_win.size else 0, OWN))
        extra_c.append(idx[idx >= WMAX])
    w = max(win_c)
    n_extra = max(len(e) for e in extra_c)

    p_iota = np.arange(128, dtype=np.int32)[:, None]
    iloc = np.arange(ROWS)
    col_lo = np.full(w, ROWS, dtype=np.int64)    # per window block, union over cores
    col_hi = np.full(w, 0, dtype=np.int64)
    mtcs, xscs = [], []
    for c in range(N_CORES):
        # Rotated mask slice (transposed) with the residual diagonal folded.
        perm = (np.arange(N) + c * ROWS) % N               # rotated row j' = source perm[j']
        mtc = np.ascontiguousarray(mask[c * ROWS:(c + 1) * ROWS, :].T[perm]
                                   ).astype(np.float16)
        mtc[iloc, iloc] += np.float16(N_NEIGHS)            # diagonal now at rows 0..ROWS-1
        mtcs.append(mtc)
        xscs.append(xs16[perm])
        nzcols = mtc[:w * 128].reshape(w, 128, ROWS).any(axis=1)   # [w, ROWS]
        for b in range(w):
            nz = np.flatnonzero(nzcols[b])
            if nz.size:
                col_lo[b] = min(col_lo[b], nz[0])
                col_hi[b] = max(col_hi[b], nz[-1] + 1)

    # Raw per-block column ranges (union over cores).
    ranges = []
    for b in range(w):
        blo, bhi = int(col_lo[b]), int(col_hi[b])
        if blo >= bhi:
            blo = bhi = 0
        ranges.append((blo, bhi))

    in_maps = []
    for c in range(N_CORES):
        mtc, xsc = mtcs[c], xscs[c]
        mw = (np.concatenate(
            [mtc[b * 128:(b + 1) * 128, lo:hi].T for b, (lo, hi) in enumerate(ranges)
             if hi > lo], axis=0).T
            if any(hi > lo for lo, hi in ranges) else np.zeros((128, 1), np.float16))
        m = {
            "mw": np.ascontiguousarray(mw),
            "xw": _pack(xsc[:w * 128]),
            "w1": w1h, "w2": w2h, "b1": b1r, "b2": b2r,
        }
        if n_extra:
            mt_full = np.zeros((N + 128, ROWS), dtype=np.float16)
            mt_full[:N] = mtc
            xs_full = np.zeros((N + 128, D_IN), dtype=np.float16)
            xs_full[:N] = xsc
            kidx = np.full(n_extra, KCH, dtype=np.int32)   # pad -> zero block
            kidx[:len(extra_c[c])] = extra_c[c]
            m["mt"] = mt_full
            m["xs"] = xs_full
            m["of"] = np.ascontiguousarray(
                (kidx[None, :] * 128 + p_iota).astype(np.int32))
        in_maps.append(m)
    return (w, n_extra, tuple(ranges)), in_maps


def kernel(x, real_edge_mask, fake_edge_mask, W1, b1, W2, b2):
    key, in_maps = _prepare_in_maps(x, fake_edge_mask, W1, b1, W2, b2)
    nc = _get_program(key)
    trace = bool(int(os.environ.get("KERNEL_TRACE", "0")))
    if trace:
        _install_ntff_hook()
    res = run_bass_kernel_spmd(nc, in_maps, list(range(N_CORES)), trace=trace)
    LAST["exec_time_ns"] = res.exec_time_ns
    LAST["results"] = res
    out = np.concatenate(
        [np.ascontiguousarray(res.results[c]["ot"].T) for c in range(N_CORES)],
        axis=0)
    return out.astype(np.float32, copy=False)


# revision 60
# speedup vs baseline: 1.0236x; 1.0236x over previous
"""Trainium2 Bass kernel for nn_MeanAddCelltype (GNN mean-aggregate + residual + MLP).

Reference semantics (N=8192 nodes, K=16 neighbors, D=512):
    idx  = top_k(fake_edge_mask, 16).indices          # per-row indices of the 16 ones
    res  = mean(x[idx], axis=1)                       # neighbor mean
    out  = relu((x + res) @ W1 + b1) @ W2 + b2

Because fake_edge_mask has exactly 16 ones per row and the neighbor sum is
permutation-invariant, res == (fake_edge_mask @ x) / 16 exactly. We compute
the aggregation as a block-sparse mask matmul on the tensor engine instead of
a top_k + gather.

Sharding: rows (nodes) are split across 8 cores, 1024 rows each; the MLP
weights are replicated. No collectives.

Block sparsity: the contraction over source nodes j (64 chunks of 128) only
matters for chunks where this core's mask slice has any nonzero. The host
scans block occupancy (CSR-style metadata, indices only).

Row rotation: each core relabels source nodes j' = (j + c*1024) mod N and
applies the same permutation to the mask rows and the x rows it contracts
against — a content-preserving relayout that leaves the output unchanged.
This puts every core's own-diagonal blocks (and, for neighborhood-local
graphs, all its occupied blocks) at block indices 0..W-1, so phase 1 reads a
statically-addressed packed window with a few large direct DMAs. Occupied
blocks beyond the window (arbitrary masks) are fetched by indirect row
gathers driven by a host-provided offset table; cores with fewer extra
blocks point the pad entries at an appended all-zero block.

Residual folding: the host adds 16*I (exact in fp16) on the core's own rows'
diagonal, which after rotation lies in window blocks 0..7. With x pre-scaled
by 1/16, the block matmul then accumulates res + x = hidden directly in
PSUM, so no separate residual add is needed.

Layout trick: all activations are kept feature-major ("transposed", [D, rows])
so every matmul consumes natural-layout operands:
    hiddenT [512,1024] = sum_{k in blocks} (x/16)[k].T-part @ (maskT+16I)[k]
    h1T  [1024,1024]   = relu(W1.T-part @ hiddenT + b1)
    outT [512,1024]    = W2.T-part @ h1T + b2
All matmul operands are fp16 (mask 0/1 and 16 are exact in fp16; x/W rounding
gives ~5e-4 rel err); accumulation is fp32 in PSUM. The host transposes
per-core mask/x slices and transposes the per-core outputs back.
"""

import os
import numpy as np

import concourse.bass as bass
import concourse.bacc as bacc
import concourse.mybir as mybir
import concourse.tile as tile
from concourse.bass_utils import run_bass_kernel_spmd

N = 8192
D_IN = 512
D_HID = 1024
D_OUT = 512
N_NEIGHS = 16
N_CORES = 8
ROWS = N // N_CORES          # 1024 rows per core
KCH = N // 128               # 64 possible contraction chunks over source nodes
OWN = ROWS // 128            # 8 diagonal blocks per core
WMAX = 16                    # max static-window size (blocks)
F16 = mybir.dt.float16
F32 = mybir.dt.float32
I32 = mybir.dt.int32
AF = mybir.ActivationFunctionType

# Results of the last hardware run (for test harness introspection).
LAST = {}

_PROGRAMS = {}


def _install_ntff_hook():
    """Best-effort shim for NTFF profiling under axon.

    This image's ``antenv`` package lacks the ``axon_hooks`` module that
    ``run_bass_kernel_spmd(trace=True)`` consults, but the actual ctypes
    profiling driver exists in ``trn_agent_boot.trn_boot``. Register it
    ourselves, and keep profile artifacts local (no remote upload).
    Failures here only disable tracing, never the run.
    """
    import sys
    import types
    try:
        try:
            from antenv import axon_hooks  # noqa: F401
            return
        except ImportError:
            pass
        import antenv
        from trn_agent_boot.trn_boot import _ntff_profile_via_ctypes
        hook = _ntff_profile_via_ctypes("/opt/axon/libaxon_pjrt.so")
        mod = types.ModuleType("antenv.axon_hooks")
        mod._hook = hook
        mod.set_axon_ntff_profile_hook = lambda h: setattr(mod, "_hook", h)
        mod.get_axon_ntff_profile_hook = lambda: mod._hook
        sys.modules["antenv.axon_hooks"] = mod
        antenv.axon_hooks = mod
        import concourse.bass_utils as bu
        bu.upload_artifacts = lambda tmpdir: "local://" + str(tmpdir)
    except Exception as e:  # pragma: no cover
        print(f"ntff hook install failed ({e!r}); tracing disabled", file=sys.stderr)


def _window_splits(w):
    """Split window blocks into DMA chunks, small first so PE starts early."""
    splits, lo, size = [], 0, 1
    while lo < w:
        hi = min(w, lo + size)
        splits.append((lo, hi))
        lo = hi
        size = 4
    return splits


def _build_program(w, n_extra, ranges):
    """Per-core Bass/Tile program (same BIR on all 8 cores): ``w`` static
    window blocks + ``n_extra`` gathered blocks in the phase-1 contraction.

    ``ranges[b]`` (window blocks only) is the (lo, hi) column range — the
    union over cores of this block's nonzero mask columns. Extra blocks
    always run full width."""
    nc = bacc.Bacc("TRN2", target_bir_lowering=False, debug=False,
                   num_devices=N_CORES)

    # Packed static window, partition-major. The mask window is packed by
    # each block's nonzero column range (``ranges[b] = (lo, hi)``); the x
    # window is dense: [p, b*D_IN + j].
    wid = [hi - lo for lo, hi in ranges]
    poff = np.concatenate([[0], np.cumsum(wid)]).tolist()        # pack offsets
    mw = nc.dram_tensor("mw", [128, max(poff[-1], 1)], F16, kind="ExternalInput")
    xw = nc.dram_tensor("xw", [128, w * D_IN], F16, kind="ExternalInput")
    if n_extra:
        # Full rotated tensors (+ one all-zero pad block) for row gathers.
        mt = nc.dram_tensor("mt", [N + 128, ROWS], F16, kind="ExternalInput")
        xs = nc.dram_tensor("xs", [N + 128, D_IN], F16, kind="ExternalInput")
        of = nc.dram_tensor("of", [128, n_extra], I32, kind="ExternalInput")
    w1 = nc.dram_tensor("w1", [D_IN, D_HID], F16, kind="ExternalInput")
    w2 = nc.dram_tensor("w2", [D_HID, D_OUT], F16, kind="ExternalInput")
    b1 = nc.dram_tensor("b1", [D_HID // 128, 128, 1], F32, kind="ExternalInput")
    b2 = nc.dram_tensor("b2", [D_OUT // 128, 128, 1], F32, kind="ExternalInput")
    ot = nc.dram_tensor("ot", [D_OUT, ROWS], F32, kind="ExternalOutput")    # outT

    w1_v = w1.ap().rearrange("(n p) m -> n p m", p=128)   # [4, 128, 1024]
    w2_v = w2.ap().rearrange("(n p) m -> n p m", p=128)   # [8, 128, 512]
    ot_v = ot.ap().rearrange("(n p) m -> n p m", p=128)   # [4, 128, 1024]

    # Per-bank-half accumulation-group stop bookkeeping: the last block
    # contributing to half h carries stop for all 4 d-accumulators' h-bank.
    blocks_h = {h: [b for b in range(w)
                    if ranges[b][0] < (h + 1) * 512 and ranges[b][1] > h * 512]
                + list(range(w, w + n_extra)) for h in (0, 1)}

    with tile.TileContext(nc) as tc:
        with (
            tc.tile_pool(name="const", bufs=1) as const,
            tc.tile_pool(name="io", bufs=3) as io,
            tc.tile_pool(name="acts", bufs=1) as acts,
            tc.tile_pool(name="acc", bufs=8, space=bass.MemorySpace.PSUM) as acc,
        ):
            # --- phase 1: hiddenT = sum_k (x/16)[k].T @ (maskT+16I)[k] ---
            # 4 PSUM accumulators, one per d-chunk, each [128, 1024] fp32 =
            # two PSUM banks. Block matmuls write partial, possibly
            # overlapping column ranges, and a matmul's PSUM range must be
            # all-pending or all-initialized — so each accumulator is armed
            # first by a full-width matmul against a zeroed moving operand
            # (start=True). These arming matmuls have no DMA dependency and
            # run during the startup window (also warming the PE clock).
            ps = [acc.tile([128, 512], F32, tag="ps", name=f"ps{g}")
                  for g in range(8)]
            zt = acts.tile([128, 512], F16, name="zt")
            nc.gpsimd.memset(zt[:], 0.0)
            for g in range(8):
                nc.tensor.matmul(ps[g][:], zt[:, :128], zt[:],
                                 start=True, stop=False,
                                 skip_group_check=True)

            def block_matmuls(b, mk, xk, mo, xo):
                # Window block: mk holds packed columns [lo, hi) at offset
                # mo. Extra block: mk holds all 1024 columns. Matmuls may
                # not cross the PSUM bank boundary, so clip at column 512.
                (blo, bhi) = ranges[b] if b < w else (0, ROWS)
                segs = [(h, max(blo, h * 512), min(bhi, (h + 1) * 512))
                        for h in range(2)]
                segs = [(h, lo, hi) for (h, lo, hi) in segs if lo < hi]
                for d in range(4):
                    lhsT = xk[:, xo + d * 128:xo + (d + 1) * 128]
                    for (h, lo, hi) in segs:
                        nc.tensor.matmul(
                            ps[d * 2 + h][:, lo - h * 512:hi - h * 512],
                            lhsT,
                            mk[:, mo + (lo - blo):mo + (hi - blo)],
                            start=False,
                            stop=(b == blocks_h[h][-1]),
                            skip_group_check=True,
                        )

            if n_extra:
                of_sb = const.tile([128, n_extra], I32, name="of_sb")
                nc.sync.dma_start(of_sb[:], of.ap()[:])

            for s, (lo, hi) in enumerate(_window_splits(w)):
                nb = hi - lo
                mwid = poff[hi] - poff[lo]
                xk = io.tile([128, nb * D_IN], F16, tag="xk", name=f"xk{s}")
                mk = io.tile([128, max(mwid, 1)], F16, tag="mk", name=f"mk{s}")
                # Launch x loads from the sync queue and mask loads from the
                # gpsimd queue so the per-sequencer launch overheads overlap.
                nc.sync.dma_start(xk[:], xw.ap()[:, lo * D_IN:hi * D_IN])
                if mwid:
                    nc.gpsimd.dma_start(mk[:], mw.ap()[:, poff[lo]:poff[hi]])
                for b in range(lo, hi):
                    block_matmuls(b, mk, xk, poff[b] - poff[lo], (b - lo) * D_IN)

            for e in range(n_extra):
                mk = io.tile([128, ROWS], F16, tag="mke", name=f"mke{e}")
                xk = io.tile([128, D_IN], F16, tag="xke", name=f"xke{e}")
                nc.gpsimd.indirect_dma_start(
                    out=mk[:], out_offset=None, in_=mt.ap(),
                    in_offset=bass.IndirectOffsetOnAxis(ap=of_sb[:, e:e + 1], axis=0),
                )
                nc.gpsimd.indirect_dma_start(
                    out=xk[:], out_offset=None, in_=xs.ap(),
                    in_offset=bass.IndirectOffsetOnAxis(ap=of_sb[:, e:e + 1], axis=0),
                )
                block_matmuls(w + e, mk, xk, 0, 0)

            # --- resident constants (needed from phase 2 on) -------------
            w1_sb = []
            for i in range(4):
                t = const.tile([128, D_HID], F16, name=f"w1_{i}")
                nc.sync.dma_start(t[:], w1_v[i])
                w1_sb.append(t)
            w2_sb = []
            for i in range(8):
                t = const.tile([128, D_OUT], F16, name=f"w2_{i}")
                nc.sync.dma_start(t[:], w2_v[i])
                w2_sb.append(t)
            b1_sb = const.tile([128, 8], F32, name="b1_sb")
            for m in range(8):
                nc.sync.dma_start(b1_sb[:, m:m + 1], b1.ap()[m])
            b2_sb = const.tile([128, 4], F32, name="b2_sb")
            for m in range(4):
                nc.sync.dma_start(b2_sb[:, m:m + 1], b2.ap()[m])

            # --- phase 2: hT = fp16(psum) -------------------------------
            # (residual already folded in via the +16I diagonal). Split the
            # PSUM->SBUF casts across DVE and ACT.
            hT = [acts.tile([128, ROWS], F16, name=f"hT{d}") for d in range(4)]
            for h in range(2):
                for d in range(4):
                    dst = hT[d][:, h * 512:(h + 1) * 512]
                    if d % 2 == 0:
                        nc.vector.tensor_copy(dst, ps[d * 2 + h][:])
                    else:
                        nc.scalar.copy(dst, ps[d * 2 + h][:])

            # --- phase 3: h1T = relu(W1_part.T @ hT + b1) ----------------
            # 1024-wide moving operands (2 PSUM banks per matmul) halve the
            # LDWEIGHTS count; evacuation still runs per 512-col half.
            h1 = [acts.tile([128, ROWS], F16, name=f"h1_{m}") for m in range(8)]
            for m in range(8):
                for h in range(2):
                    pg = acc.tile([128, 512], F32, tag="ps", name=f"pg1_{m}_{h}")
                    for kd in range(4):
                        nc.tensor.matmul(
                            pg[:],
                            w1_sb[kd][:, m * 128:(m + 1) * 128],
                            hT[kd][:, h * 512:(h + 1) * 512],
                            start=(kd == 0),
                            stop=(kd == 3),
                        )
                    nc.scalar.activation(
                        h1[m][:, h * 512:(h + 1) * 512], pg[:],
                        AF.Relu, bias=b1_sb[:, m:m + 1],
                    )

            # --- phase 4: outT = W2_part.T @ h1T + b2 --------------------
            for o in range(4):
                ob = acts.tile([128, ROWS], F32, name=f"ob{o}")
                for h in range(2):
                    pg = acc.tile([128, 512], F32, tag="ps", name=f"pg2_{o}_{h}")
                    for kh in range(8):
                        nc.tensor.matmul(
                            pg[:],
                            w2_sb[kh][:, o * 128:(o + 1) * 128],
                            h1[kh][:, h * 512:(h + 1) * 512],
                            start=(kh == 0),
                            stop=(kh == 7),
                        )
                    nc.scalar.activation(
                        ob[:, h * 512:(h + 1) * 512], pg[:],
                        AF.Identity, bias=b2_sb[:, o:o + 1],
                    )
                    nc.sync.dma_start(ot_v[o][:, h * 512:(h + 1) * 512],
                                      ob[:, h * 512:(h + 1) * 512])

    nc.compile()
    return nc


def _get_program(key):
    if key not in _PROGRAMS:
        _PROGRAMS[key] = _build_program(*key)
    return _PROGRAMS[key]


def _pack(v):
    """[nb*128, fd] chunk-major -> [128, nb*fd] partition-major packing."""
    nb = v.shape[0] // 128
    return np.ascontiguousarray(
        v.reshape(nb, 128, v.shape[1]).transpose(1, 0, 2)).reshape(128, -1)


def _effective_mask(mask):
    """Reproduce top_k(mask, 16) selection semantics exactly: the reference
    gathers the 16 highest-valued columns per row with ties broken by
    ascending index. For rows with exactly 16 ones (the documented
    invariant) that is just the ones; rows that deviate select the
    lowest-index ones first, then the lowest-index zeros. No-op cost when
    every row has exactly 16 ones."""
    cnt = mask.sum(axis=1)
    bad = np.flatnonzero(cnt != N_NEIGHS)
    if not bad.size:
        return mask
    mask = mask.copy()
    for r in bad:
        ones = np.flatnonzero(mask[r])
        sel = ones[:N_NEIGHS]
        if sel.size < N_NEIGHS:
            zeros = np.flatnonzero(~mask[r])
            sel = np.concatenate([sel, zeros[:N_NEIGHS - sel.size]])
        row = np.zeros(mask.shape[1], dtype=bool)
        row[sel] = True
        mask[r] = row
    return mask


def _prepare_in_maps(x, fake_edge_mask, W1, b1, W2, b2):
    x = np.asarray(x, dtype=np.float32)
    mask = _effective_mask(np.asarray(fake_edge_mask).astype(bool))
    xs16 = (x * (1.0 / N_NEIGHS)).astype(np.float16)       # exact pow2 scale
    w1h = np.asarray(W1, dtype=np.float32).astype(np.float16)
    w2h = np.asarray(W2, dtype=np.float32).astype(np.float16)
    b1r = np.ascontiguousarray(
        np.asarray(b1, dtype=np.float32).reshape(D_HID // 128, 128, 1))
    b2r = np.ascontiguousarray(
        np.asarray(b2, dtype=np.float32).reshape(D_OUT // 128, 128, 1))

    # Occupied 128-row source blocks per core in ROTATED order (indices-only
    # metadata). Rotation: core c relabels source j -> (j - c*ROWS) mod N,
    # which is a left-rotation of blocks by c*OWN. The +16I diagonal then
    # occupies blocks 0..OWN-1 (always in-window).
    occ = mask.reshape(N_CORES, ROWS, KCH, 128).any(axis=(1, 3))
    win_c, extra_c = [], []
    for c in range(N_CORES):
        occ_rot = np.roll(occ[c], -c * OWN)
        idx = np.flatnonzero(occ_rot)
        in_win = idx[idx < WMAX]
        win_c.append(max(int(in_win.max()) + 1 if in_win.size else 0, OWN))
        extra_c.append(idx[idx >= WMAX])
    w = max(win_c)
    n_extra = max(len(e) for e in extra_c)

    p_iota = np.arange(128, dtype=np.int32)[:, None]
    iloc = np.arange(ROWS)
    col_lo = np.full(w, ROWS, dtype=np.int64)    # per window block, union over cores
    col_hi = np.full(w, 0, dtype=np.int64)
    mtcs, xscs = [], []
    for c in range(N_CORES):
        # Rotated mask slice (transposed) with the residual diagonal folded.
        perm = (np.arange(N) + c * ROWS) % N               # rotated row j' = source perm[j']
        mtc = np.ascontiguousarray(mask[c * ROWS:(c + 1) * ROWS, :].T[perm]
                                   ).astype(np.float16)
        mtc[iloc, iloc] += np.float16(N_NEIGHS)            # diagonal now at rows 0..ROWS-1
        mtcs.append(mtc)
        xscs.append(xs16[perm])
        nzcols = mtc[:w * 128].reshape(w, 128, ROWS).any(axis=1)   # [w, ROWS]
        for b in range(w):
            nz = np.flatnonzero(nzcols[b])
            if nz.size:
                col_lo[b] = min(col_lo[b], nz[0])
                col_hi[b] = max(col_hi[b], nz[-1] + 1)

    # Raw per-block column ranges (union over cores).
    ranges = []
    for b in range(w):
        blo, bhi = int(col_lo[b]), int(col_hi[b])
        if blo >= bhi:
            blo = bhi = 0
        ranges.append((blo, bhi))

    in_maps = []
    for c in range(N_CORES):
        mtc, xsc = mtcs[c], xscs[c]
        mw = (np.concatenate(
            [mtc[b * 128:(b + 1) * 128, lo:hi].T for b, (lo, hi) in enumerate(ranges)
             if hi > lo], axis=0).T
            if any(hi > lo for lo, hi in ranges) else np.zeros((128, 1), np.float16))
        m = {
            "mw": np.ascontiguousarray(mw),
            "xw": _pack(xsc[:w * 128]),
            "w1": w1h, "w2": w2h, "b1": b1r, "b2": b2r,
        }
        if n_extra:
            mt_full = np.zeros((N + 128, ROWS), dtype=np.float16)
            mt_full[:N] = mtc
            xs_full = np.zeros((N + 128, D_IN), dtype=np.float16)
            xs_full[:N] = xsc
            kidx = np.full(n_extra, KCH, dtype=np.int32)   # pad -> zero block
            kidx[:len(extra_c[c])] = extra_c[c]
            m["mt"] = mt_full
            m["xs"] = xs_full
            m["of"] = np.ascontiguousarray(
                (kidx[None, :] * 128 + p_iota).astype(np.int32))
        in_maps.append(m)
    return (w, n_extra, tuple(ranges)), in_maps


def kernel(x, real_edge_mask, fake_edge_mask, W1, b1, W2, b2):
    key, in_maps = _prepare_in_maps(x, fake_edge_mask, W1, b1, W2, b2)
    nc = _get_program(key)
    trace = bool(int(os.environ.get("KERNEL_TRACE", "0")))
    if trace:
        _install_ntff_hook()
    res = run_bass_kernel_spmd(nc, in_maps, list(range(N_CORES)), trace=trace)
    LAST["exec_time_ns"] = res.exec_time_ns
    LAST["results"] = res
    out = np.concatenate(
        [np.ascontiguousarray(res.results[c]["ot"].T) for c in range(N_CORES)],
        axis=0)
    return out.astype(np.float32, copy=False)
